# revision 10
# baseline (speedup 1.0000x reference)
"""Trainium2 Bass kernel for the Conv-RBS density-matrix problem.

Math: the reference applies 18 RBS (Givens) gates sequentially as
rho <- U rho U^T.  Conjugations compose, and every gate factorizes over
the (row=16, col=16, ch=4) tensor-product structure of the 1024-dim
space, so the whole scan collapses to

    out = W @ rho @ W.T,   W = R (x) C (x) H

with R, C 16x16 rotations that are identity outside their top-left 4x4
block, and H a dense 4x4 rotation.  In 128x128 blocks W is
block-diagonal with B = I2 (x) C (x) H everywhere except a 2x2 block
grid in the top-left 256x256 corner (R's 4x4 block).

Device scheme (per core c of 8, column-sharded on rho):
  pass A:  ZT[c,I] = sum_K  rho[K,c]^T @ W[I,K]^T     (TensorE: lhsT=rho
           block, rhs=const) -- i.e. the c-th block-row of (W rho)^T.
  pass B:  out[I,J] += ZT[c,I]^T @ W[J,c]^T           (lhsT=ZT block)
Cores c>=2 produce the finished column slab out[:,c]; cores 0,1 produce
partial sums for columns 0..255 which the host adds.  No transposes and
no collectives are needed anywhere.
"""

import contextlib
import ctypes
import os
import sys
import types

import numpy as np

import concourse.bass as bass
import concourse.mybir as mybir
from concourse import bacc, bass_utils
from concourse.tile import TileContext


def _install_axon_ntff_shim():
    """The agent image's ``antenv`` lacks ``axon_hooks``, so bass_utils'
    trace=True path crashes instead of profiling.  Recreate the hook the
    way trn_boot would have: ctypes into libaxon_pjrt.so."""
    try:
        import antenv.axon_hooks  # noqa: F401
        return
    except ImportError:
        pass
    so_path = "/opt/axon/libaxon_pjrt.so"
    hook = None
    if os.path.exists(so_path):
        try:
            lib = ctypes.CDLL(so_path)
            lib.axon_start_nrt_profile.argtypes = [
                ctypes.POINTER(ctypes.c_int64),
                ctypes.c_size_t,
            ]
            lib.axon_start_nrt_profile.restype = ctypes.c_int64
            lib.axon_stop_nrt_profile.argtypes = [ctypes.c_char_p]
            lib.axon_stop_nrt_profile.restype = ctypes.c_int64

            @contextlib.contextmanager
            def _hook(output_dir, device_ids):
                import jax

                jax.devices()
                if device_ids:
                    ids = (ctypes.c_int64 * len(device_ids))(*device_ids)
                    rc = lib.axon_start_nrt_profile(ids, len(device_ids))
                else:
                    rc = lib.axon_start_nrt_profile(None, 0)
                if rc != 0:
                    raise RuntimeError(f"axon_start_nrt_profile rc={rc}")
                try:
                    yield
                finally:
                    n = lib.axon_stop_nrt_profile(str(output_dir).encode())
                    if n < 0:
                        raise RuntimeError(f"axon_stop_nrt_profile rc={n}")

            hook = _hook
        except (OSError, AttributeError):
            hook = None

    mod = types.ModuleType("antenv.axon_hooks")
    mod.get_axon_ntff_profile_hook = lambda: hook
    mod.set_axon_ntff_profile_hook = lambda h: None
    sys.modules["antenv.axon_hooks"] = mod


def _patch_upload_artifacts():
    """Artifact upload needs bucket creds this container may not have;
    don't let a failed upload kill the profiled run."""
    orig = bass_utils.upload_artifacts
    if getattr(orig, "_safe_wrapped", False):
        return

    def safe_upload(tmpdir):
        try:
            return orig(tmpdir)
        except Exception:
            return tmpdir

    safe_upload._safe_wrapped = True
    bass_utils.upload_artifacts = safe_upload


_install_axon_ntff_shim()
_patch_upload_artifacts()

I_DIM, J_DIM, KGATE = 16, 4, 4
D = 1024
P = 128
NCORES = 8

LAST_EXEC_NS = None  # filled when BASS_TRACE is set


def _gate_list():
    gates = []
    for i in range(KGATE):
        for j in range(i + 1, KGATE):
            gates.append(("row", i, j))
    for i in range(KGATE):
        for j in range(i + 1, KGATE):
            gates.append(("col", i, j))
    for i in range(J_DIM):
        for j in range(i + 1, J_DIM):
            gates.append(("ch", i, j))
    return gates


def _build_w_blocks(thetas):
    """Return (BT, PA0, PA1, RHSB[8]) fp32 host constants."""
    mats = {"row": np.eye(I_DIM), "col": np.eye(I_DIM), "ch": np.eye(J_DIM)}
    for (reg, a, b), th in zip(_gate_list(), np.asarray(thetas, dtype=np.float64)):
        n = mats[reg].shape[0]
        G = np.eye(n)
        c, s = np.cos(th), np.sin(th)
        G[a, a] = c
        G[b, b] = c
        G[a, b] = s
        G[b, a] = -s
        mats[reg] = G @ mats[reg]
    R, C, H = mats["row"], mats["col"], mats["ch"]
    Q = np.kron(C, H)  # 64x64
    B = np.kron(np.eye(2), Q)  # 128x128, W[I,I] for I>=2
    # top-left 2x2 block grid: W[i,k] = R[2i:2i+2, 2k:2k+2] (x) Q
    Wtop = [[np.kron(R[2 * i : 2 * i + 2, 2 * k : 2 * k + 2], Q) for k in range(2)]
            for i in range(2)]
    BT = B.T
    # pass-A packed rhs for I<2:  PA[K] = [ W[0,K]^T | W[1,K]^T ]
    PA0 = np.concatenate([Wtop[0][0].T, Wtop[1][0].T], axis=1)
    PA1 = np.concatenate([Wtop[0][1].T, Wtop[1][1].T], axis=1)
    # pass-B rhs per core:  c<2 -> [ W[0,c]^T | W[1,c]^T ],  c>=2 -> [ B^T | 0 ]
    rhsb = []
    for c in range(NCORES):
        if c < 2:
            rhsb.append(np.concatenate([Wtop[0][c].T, Wtop[1][c].T], axis=1))
        else:
            rhsb.append(np.concatenate([BT, np.zeros_like(BT)], axis=1))
    f32 = np.float32
    return (
        np.ascontiguousarray(BT, dtype=f32),
        np.ascontiguousarray(PA0, dtype=f32),
        np.ascontiguousarray(PA1, dtype=f32),
        [np.ascontiguousarray(r, dtype=f32) for r in rhsb],
    )


BLOB_W = 1024 + 640 + 256  # rho slab (K-major) | BT,PA0,PA1 | rhs_b


def _build_program():
    f32 = mybir.dt.float32
    nc = bacc.Bacc(None)
    blob_in = nc.declare_dram_parameter("blob", [P, BLOB_W], f32, isOutput=False)
    outp = nc.declare_dram_parameter("outp", [D, 256], f32, isOutput=True)

    with TileContext(nc) as tc:
        with (
            tc.tile_pool(name="const", bufs=1) as cpool,
            tc.tile_pool(name="psum", bufs=2, space="PSUM") as ppool,
        ):
            blob = cpool.tile([P, BLOB_W], f32)
            nc.sync.dma_start(blob[:], blob_in[:])
            zt = cpool.tile([P, 8, P], f32)

            rho = [blob[:, K * P : (K + 1) * P] for K in range(8)]
            BT = blob[:, 1024:1152]
            PA = [blob[:, 1152:1408], blob[:, 1408:1664]]
            rb = blob[:, 1664:1920]

            # pass A: ZT[c,I] = rho[I,c]^T @ W[I,I]^T  (plus top-corner terms)
            for I in range(2, 8):
                ps = ppool.tile([P, P], f32, tag="psA")
                nc.tensor.matmul(ps[:], rho[I], BT, start=True, stop=True)
                nc.vector.tensor_copy(out=zt[:, I, :], in_=ps[:])
            ps2 = ppool.tile([P, 256], f32, tag="psA2")
            nc.tensor.matmul(ps2[:], rho[0], PA[0], start=True, stop=False)
            nc.tensor.matmul(ps2[:], rho[1], PA[1], start=False, stop=True)
            nc.vector.tensor_copy(out=zt[:, 0, :], in_=ps2[:, 0:128])
            nc.vector.tensor_copy(out=zt[:, 1, :], in_=ps2[:, 128:256])

            # pass B: out[I, :] = ZT[c,I]^T @ rhs_b
            # outputs DMA'd in pairs: 1 input + 4 output DMAs stay on
            # distinct HW queues (a queue wrap would add a second wait,
            # which DMA instructions can't encode)
            out_t = outp.rearrange("(I p) n -> p I n", p=P)
            for pair in range(4):
                ob = cpool.tile([P, 2, 256], f32, tag=f"ob{pair}")
                for j in range(2):
                    I = 2 * pair + j
                    ps = ppool.tile([P, 256], f32, tag="psB")
                    nc.tensor.matmul(ps[:], zt[:, I, :], rb, start=True, stop=True)
                    nc.vector.tensor_copy(out=ob[:, j, :], in_=ps[:])
                nc.sync.dma_start(out_t[:, 2 * pair : 2 * pair + 2, :], ob[:])
    if not nc.is_finalized():
        nc.finalize()
    return nc


def _run(inputs, trace=False):
    global LAST_EXEC_NS
    rho = np.ascontiguousarray(np.asarray(inputs["input_state"], dtype=np.float32))
    thetas = np.asarray(inputs["thetas"], dtype=np.float32)
    BT, PA0, PA1, rhsb = _build_w_blocks(thetas)
    consts_a = np.concatenate([BT, PA0, PA1], axis=1)

    nc = _build_program()
    in_maps = []
    for c in range(NCORES):
        slab = rho[:, c * P : (c + 1) * P].reshape(8, P, P)
        rho_km = np.transpose(slab, (1, 0, 2)).reshape(P, 1024)
        blob = np.concatenate([rho_km, consts_a, rhsb[c]], axis=1)
        in_maps.append({"blob": np.ascontiguousarray(blob)})
    res = bass_utils.run_bass_kernel_spmd(
        nc, in_maps, list(range(NCORES)), trace=trace
    )
    LAST_EXEC_NS = res.exec_time_ns

    out = np.empty((D, D), dtype=np.float32)
    for c in range(2, NCORES):
        out[:, c * P : (c + 1) * P] = res.results[c]["outp"][:, :P]
    out[:, 0:256] = res.results[0]["outp"] + res.results[1]["outp"]
    return out


def kernel(**inputs):
    return _run(inputs)


# revision 13
# speedup vs baseline: 1.0822x; 1.0822x over previous
"""Trainium2 Bass kernel for the Conv-RBS density-matrix problem.

Math: the reference applies 18 RBS (Givens) gates sequentially as
rho <- U rho U^T.  Conjugations compose, and every gate factorizes over
the (row=16, col=16, ch=4) tensor-product structure of the 1024-dim
space, so the whole scan collapses to

    out = W @ rho @ W.T,   W = R (x) C (x) H

with R, C 16x16 rotations that are identity outside their top-left 4x4
block, and H a dense 4x4 rotation.  In 128x128 blocks W is
block-diagonal with B = I2 (x) C (x) H everywhere except a 2x2 block
grid in the top-left 256x256 corner (R's 4x4 block).

Device scheme (per core c of 8, column-sharded on rho):
  pass A:  ZT[c,I] = sum_K  rho[K,c]^T @ W[I,K]^T     (TensorE: lhsT=rho
           block, rhs=const) -- i.e. the c-th block-row of (W rho)^T.
  pass B:  out[I,J] += ZT[c,I]^T @ W[J,c]^T           (lhsT=ZT block)
Cores c>=2 produce the finished column slab out[:,c]; cores 0,1 produce
partial sums for columns 0..255 which the host adds.  No transposes and
no collectives are needed anywhere.
"""

import contextlib
import ctypes
import os
import sys
import types

import numpy as np

import concourse.bass as bass
import concourse.mybir as mybir
from concourse import bacc, bass_utils
from concourse.tile import TileContext


def _install_axon_ntff_shim():
    """The agent image's ``antenv`` lacks ``axon_hooks``, so bass_utils'
    trace=True path crashes instead of profiling.  Recreate the hook the
    way trn_boot would have: ctypes into libaxon_pjrt.so."""
    try:
        import antenv.axon_hooks  # noqa: F401
        return
    except ImportError:
        pass
    so_path = "/opt/axon/libaxon_pjrt.so"
    hook = None
    if os.path.exists(so_path):
        try:
            lib = ctypes.CDLL(so_path)
            lib.axon_start_nrt_profile.argtypes = [
                ctypes.POINTER(ctypes.c_int64),
                ctypes.c_size_t,
            ]
            lib.axon_start_nrt_profile.restype = ctypes.c_int64
            lib.axon_stop_nrt_profile.argtypes = [ctypes.c_char_p]
            lib.axon_stop_nrt_profile.restype = ctypes.c_int64

            @contextlib.contextmanager
            def _hook(output_dir, device_ids):
                import jax

                jax.devices()
                if device_ids:
                    ids = (ctypes.c_int64 * len(device_ids))(*device_ids)
                    rc = lib.axon_start_nrt_profile(ids, len(device_ids))
                else:
                    rc = lib.axon_start_nrt_profile(None, 0)
                if rc != 0:
                    raise RuntimeError(f"axon_start_nrt_profile rc={rc}")
                try:
                    yield
                finally:
                    n = lib.axon_stop_nrt_profile(str(output_dir).encode())
                    if n < 0:
                        raise RuntimeError(f"axon_stop_nrt_profile rc={n}")

            hook = _hook
        except (OSError, AttributeError):
            hook = None

    mod = types.ModuleType("antenv.axon_hooks")
    mod.get_axon_ntff_profile_hook = lambda: hook
    mod.set_axon_ntff_profile_hook = lambda h: None
    sys.modules["antenv.axon_hooks"] = mod


def _patch_upload_artifacts():
    """Artifact upload needs bucket creds this container may not have;
    don't let a failed upload kill the profiled run."""
    orig = bass_utils.upload_artifacts
    if getattr(orig, "_safe_wrapped", False):
        return

    def safe_upload(tmpdir):
        try:
            return orig(tmpdir)
        except Exception:
            return tmpdir

    safe_upload._safe_wrapped = True
    bass_utils.upload_artifacts = safe_upload


_install_axon_ntff_shim()
_patch_upload_artifacts()

I_DIM, J_DIM, KGATE = 16, 4, 4
D = 1024
P = 128
NCORES = 8

LAST_EXEC_NS = None  # filled when BASS_TRACE is set


def _gate_list():
    gates = []
    for i in range(KGATE):
        for j in range(i + 1, KGATE):
            gates.append(("row", i, j))
    for i in range(KGATE):
        for j in range(i + 1, KGATE):
            gates.append(("col", i, j))
    for i in range(J_DIM):
        for j in range(i + 1, J_DIM):
            gates.append(("ch", i, j))
    return gates


def _build_w_blocks(thetas):
    """Return (BT, PA0, PA1, RHSB[8]) fp32 host constants."""
    mats = {"row": np.eye(I_DIM), "col": np.eye(I_DIM), "ch": np.eye(J_DIM)}
    for (reg, a, b), th in zip(_gate_list(), np.asarray(thetas, dtype=np.float64)):
        n = mats[reg].shape[0]
        G = np.eye(n)
        c, s = np.cos(th), np.sin(th)
        G[a, a] = c
        G[b, b] = c
        G[a, b] = s
        G[b, a] = -s
        mats[reg] = G @ mats[reg]
    R, C, H = mats["row"], mats["col"], mats["ch"]
    Q = np.kron(C, H)  # 64x64
    B = np.kron(np.eye(2), Q)  # 128x128, W[I,I] for I>=2
    # top-left 2x2 block grid: W[i,k] = R[2i:2i+2, 2k:2k+2] (x) Q
    Wtop = [[np.kron(R[2 * i : 2 * i + 2, 2 * k : 2 * k + 2], Q) for k in range(2)]
            for i in range(2)]
    BT = B.T
    # pass-A packed rhs for I<2:  PA[K] = [ W[0,K]^T | W[1,K]^T ]
    PA0 = np.concatenate([Wtop[0][0].T, Wtop[1][0].T], axis=1)
    PA1 = np.concatenate([Wtop[0][1].T, Wtop[1][1].T], axis=1)
    # pass-B rhs per core:  c<2 -> [ W[0,c]^T | W[1,c]^T ],  c>=2 -> [ B^T | 0 ]
    rhsb = []
    for c in range(NCORES):
        if c < 2:
            rhsb.append(np.concatenate([Wtop[0][c].T, Wtop[1][c].T], axis=1))
        else:
            rhsb.append(np.concatenate([BT, np.zeros_like(BT)], axis=1))
    f32 = np.float32
    return (
        np.ascontiguousarray(BT, dtype=f32),
        np.ascontiguousarray(PA0, dtype=f32),
        np.ascontiguousarray(PA1, dtype=f32),
        [np.ascontiguousarray(r, dtype=f32) for r in rhsb],
    )


# blob columns: BT | PA0 | PA1 | RB | rho K-major (pass-A order 2..7,0,1)
BLOB_W = 640 + 256 + 1024
RHO0 = 896  # column offset of first rho block in blob
RHO_ORDER = [2, 3, 4, 5, 6, 7, 0, 1]


def _build_program():
    f32 = mybir.dt.float32
    nc = bacc.Bacc(None)
    blob_in = nc.declare_dram_parameter("blob", [P, BLOB_W], f32, isOutput=False)
    # partition-major output: outp[p, I*256+n] = out[I*128+p, n] -- keeps
    # DMA descriptors at 8KB instead of 1KB; host un-transposes
    outp = nc.declare_dram_parameter("outp", [P, 8 * 256], f32, isOutput=True)

    with TileContext(nc) as tc:
        with (
            tc.tile_pool(name="const", bufs=1) as cpool,
            tc.tile_pool(name="psum", bufs=2, space="PSUM") as ppool,
        ):
            blob = cpool.tile([P, BLOB_W], f32)
            # consts + rhs_b first, then rho in compute order: compute can
            # start as soon as the first pieces land
            nc.sync.dma_start(blob[:, 0:RHO0], blob_in[:, 0:RHO0])
            for d in range(4):
                cols = slice(RHO0 + d * 256, RHO0 + (d + 1) * 256)
                nc.sync.dma_start(blob[:, cols], blob_in[:, cols])

            zt = cpool.tile([P, 8, P], f32)
            rho = {}
            for pos, K in enumerate(RHO_ORDER):
                rho[K] = blob[:, RHO0 + pos * P : RHO0 + (pos + 1) * P]
            BT = blob[:, 0:128]
            PA = [blob[:, 128:384], blob[:, 384:640]]
            rb = blob[:, 640:896]

            def pass_b(I):
                ps = ppool.tile([P, 256], f32, tag="psB")
                nc.tensor.matmul(ps[:], zt[:, I, :], rb, start=True, stop=True)
                ob = cpool.tile([P, 256], f32, tag=f"ob{I}")
                nc.vector.tensor_copy(out=ob[:], in_=ps[:])
                nc.sync.dma_start(outp[:, I * 256 : (I + 1) * 256], ob[:])

            # interleave pass A / pass B per block: B(I) only needs ZT[c,I]
            for I in range(2, 8):
                ps = ppool.tile([P, P], f32, tag="psA")
                nc.tensor.matmul(ps[:], rho[I], BT, start=True, stop=True)
                nc.vector.tensor_copy(out=zt[:, I, :], in_=ps[:])
                pass_b(I)
            ps2 = ppool.tile([P, 256], f32, tag="psA2")
            nc.tensor.matmul(ps2[:], rho[0], PA[0], start=True, stop=False)
            nc.tensor.matmul(ps2[:], rho[1], PA[1], start=False, stop=True)
            nc.vector.tensor_copy(out=zt[:, 0, :], in_=ps2[:, 0:128])
            nc.vector.tensor_copy(out=zt[:, 1, :], in_=ps2[:, 128:256])
            pass_b(0)
            pass_b(1)
    if not nc.is_finalized():
        nc.finalize()
    return nc


def _run(inputs, trace=False):
    global LAST_EXEC_NS
    rho = np.ascontiguousarray(np.asarray(inputs["input_state"], dtype=np.float32))
    thetas = np.asarray(inputs["thetas"], dtype=np.float32)
    BT, PA0, PA1, rhsb = _build_w_blocks(thetas)
    consts_a = np.concatenate([BT, PA0, PA1], axis=1)

    nc = _build_program()
    in_maps = []
    for c in range(NCORES):
        slab = rho[:, c * P : (c + 1) * P].reshape(8, P, P)
        rho_km = np.transpose(slab[RHO_ORDER], (1, 0, 2)).reshape(P, 1024)
        blob = np.concatenate([consts_a, rhsb[c], rho_km], axis=1)
        in_maps.append({"blob": np.ascontiguousarray(blob)})
    res = bass_utils.run_bass_kernel_spmd(
        nc, in_maps, list(range(NCORES)), trace=trace
    )
    LAST_EXEC_NS = res.exec_time_ns

    out = np.empty((D, D), dtype=np.float32)

    def unpack(c):
        # outp[p, I*256+n] -> [1024, 256]
        return (
            res.results[c]["outp"]
            .reshape(P, 8, 256)
            .transpose(1, 0, 2)
            .reshape(D, 256)
        )

    for c in range(2, NCORES):
        out[:, c * P : (c + 1) * P] = unpack(c)[:, :P]
    out[:, 0:256] = unpack(0) + unpack(1)
    return out


def kernel(**inputs):
    return _run(inputs)
